# revision 16
# baseline (speedup 1.0000x reference)
"""Trainium2 Bass kernel for nn_BasicTransformerBlock_12738873000028.

Strategy (8 NeuronCores): data-parallel over batch (2) x sequence-parallel over
query rows (4) => core c handles batch c//4, query rows [(c%4)*1024, +1024).

v2: fp8 (e4m3) DoubleRow matmuls + projection folding.

- K-projection is folded into the Q projection: scores = (x Wq + bq)·(k Wk + bk)
  = x (Wq Wk^T) k^T + (Wk bq)·k + const(n)  -- the bk term is constant per query
  and cancels in softmax, so keys are the RAW x / context (already resident in
  fp8), and the fused Q weight is Wq @ Wk.T with bias row Wk @ bq.
- V-projection is folded into the output projection: (sum_m s_m v_m) Wp =
  (sum_m s_m x_m)(Wv Wp) + (bv Wp + bp), so V is the raw x / context in
  token-major fp8 layout (host-prepped) with a ones column providing the
  softmax denominator; denominator * reciprocal == 1 doubles as the bias-row
  input for the fused P projection.
- All big matmuls are fp8 with DoubleRow perf mode over channel rows 0..255
  ([128,2] interleave) plus a plain fp8 matmul for the 64/65-row tail (small-K
  DoubleRow measured 3x slower than plain -- avoid).
- Weights are host-scaled (QK-fused x1024*C^-0.5, VP-fused x1024, FFN x64) to
  sit in fp8's normal range; descales fold into the exp scale, the softmax
  reciprocal path, and scalar_tensor_tensor residual updates. The fp32
  residual stream, group-norm statistics and the 16x2 AllReduce are unchanged.
"""

import numpy as np
import ml_dtypes

import concourse.bacc as bacc
import concourse.tile as tile
from concourse import mybir
from concourse.bass_utils import run_bass_kernel_spmd

bf16 = ml_dtypes.bfloat16
fp8 = ml_dtypes.float8_e4m3
F32 = mybir.dt.float32
BF16 = mybir.dt.bfloat16
FP8 = mybir.dt.float8e4
DR = mybir.MatmulPerfMode.DoubleRow

B, H, W, C = 2, 64, 64, 320
N = H * W                      # 4096 tokens per batch
NCORES = 8
QL = N // 4                    # 1024 local query rows per core
QWIN = 512                     # query window (fp32 PSUM bank = 512 floats)
NQW = QL // QWIN               # 2 windows
C8 = 8 * C                     # 2560
C4 = 4 * C                     # 1280
GROUPS, EPS = 16, 1e-3
GSIZE = C // GROUPS            # 20 channels per group
GCNT = float(N * GSIZE)        # elements per (batch, group)
MT = N // 128                  # 32 key tiles
NPAIR = MT // 2                # 16 key-tile pairs
NDP = C4 // 256                # 5 dense contraction pairs
SCALE = float(C) ** -0.5
QS = 1024.0                    # QK-fused weight scale
PS = 1024.0                    # VP-fused weight scale
WS = 64.0                      # FFN weight scale
TOKW = 336                     # channels + ones col + pad to 16-elem stride
                               # (dual-fp8 ldweights: group stride % 16 == 0)

# output channel chunks: (offset, size)
CT = [(0, 128), (128, 128), (256, 64)]
# accumulator partition sizes (chunk2 carries denominator row + pad; even for
# dual-fp8 ldweights restrictions)
ASZ = [128, 128, 66]

_NC_CACHE = {}
_PROBE = set()    # timing probes: "pe_only" strips ACT/DVE from hot paths


def _emit_loads(nc, ap, pools):
    """Resident loads: emitted ONCE per program; bodies only read these."""
    res = pools["res"]

    def rtile(shape, dtype, tag):
        return res.tile(shape, dtype, tag=tag, name=tag)

    T = {}
    # ---------------- resident loads ----------------
    # Pure-DoubleRow layout: contraction padded 320/321 -> 512 with zeros in
    # the "b" tiles (zero stationary rows make the pad free; avoids the
    # ~800ns/pair PE penalty for alternating dual-fp8 and plain matmuls).
    T["XK1"] = rtile([128, 2, N], FP8, "xk1")     # x channels 0..255, DR pairs
    T["XK1B"] = rtile([128, 2, N], FP8, "xk1b")   # chs 256..319 in g0 + zeros
    T["CK1"] = rtile([128, 2, N], FP8, "ck1")
    T["CK1B"] = rtile([128, 2, N], FP8, "ck1b")
    T["XTOK"] = rtile([128, NPAIR, 2, TOKW], FP8, "xtok")  # token-major + ones col
    T["CTOK"] = rtile([128, NPAIR, 2, TOKW], FP8, "ctok")
    T["XQ1"] = rtile([128, 2, QL], FP8, "xq1")    # local query cols
    T["XQB"] = rtile([128, 2, QL], FP8, "xqb")    # chs 256..319 + ones + zeros
    for name in ["xk1", "xk1b", "ck1", "ck1b", "xtok", "ctok", "xq1", "xqb"]:
        nc.sync.dma_start(out=T[name.upper()], in_=ap[name])

    def load_w(name, shape):
        t = rtile(shape, FP8, f"w_{name}")
        nc.sync.dma_start(out=t, in_=ap[name])
        return t

    T["XFB"], T["GB"], T["INDsb"] = [], [], []
    for i, (off, sz) in enumerate(CT):
        t = rtile([sz, QL], BF16, f"xfb_{i}")    # 2*x local, bf16 (resid1 base)
        nc.sync.dma_start(out=t, in_=ap["xf16"][off:off + sz, :])
        T["XFB"].append(t)
        t = rtile([sz, 2], F32, f"gb_{i}")
        nc.sync.dma_start(out=t, in_=ap["gb"][off:off + sz, :])
        T["GB"].append(t)
        t = rtile([sz, GROUPS], F32, f"ind_{i}")
        nc.sync.dma_start(out=t, in_=ap["ind"][off:off + sz, :])
        T["INDsb"].append(t)
    T["INDT"] = rtile([GROUPS, C], F32, "indt")
    nc.sync.dma_start(out=T["INDT"], in_=ap["indt"][:, :])


    for nm in ["w1sa1", "wpsa1", "w1ca1", "wpca1",
               "w1sab", "wpsab", "w1cab", "wpcab"]:
        T[nm] = load_w(nm, [128, 2, C])
    T["GW"] = []
    for i, asz in enumerate([128, 128, 65]):
        t = rtile([asz, C8], BF16, f"w_gw_{i}")
        nc.sync.dma_start(out=t, in_=ap["gw"][128 * i:128 * i + asz, :])
        T["GW"].append(t)
    T["DW"] = []
    for j in range(C4 // 128):
        t = rtile([128, C], BF16, f"w_dw_{j}")
        nc.sync.dma_start(out=t, in_=ap["dw"][j * 128:(j + 1) * 128, :])
        T["DW"].append(t)
    T["DB"] = rtile([1, C], BF16, "w_db")
    nc.sync.dma_start(out=T["DB"], in_=ap["db"])

    T["ONES16W"] = rtile([1, QWIN], BF16, "ones16w")
    nc.vector.memset(T["ONES16W"], 1.0)
    # pre-zero ring-buffered pad tiles (bodies only write their live rows,
    # the zero pad persists across bodies)
    for _ in range(2):
        t = res.tile([128, 2, QWIN], FP8, tag="atb", name="atb", bufs=2)
        nc.vector.memset(t, 0.0)
    for _ in range(2):
        t = res.tile([128, 2, QL], FP8, tag="qab", name="qab", bufs=2)
        nc.vector.memset(t, 0.0)
    t = res.tile([128, 2, QL], FP8, tag="xnb", name="xnb", bufs=1)
    nc.vector.memset(t, 0.0)
    nc.vector.memset(t[64:65, 0, :], 1.0)
    return T


def _emit_body(nc, tc, ap, T, pools, upto=99, accum_out=False):
    """Emit one forward pass reading resident tiles T."""
    res, dram = pools["res"], pools["dram"]
    ps_acc2, ps_acc1 = pools["acc2"], pools["acc1"]
    ps_psc2, ps_tiny = pools["psc2"], pools["tiny"]

    XK1, XK1B, CK1, CK1B = T["XK1"], T["XK1B"], T["CK1"], T["CK1B"]
    XTOK, CTOK, XQ1, XQB = T["XTOK"], T["CTOK"], T["XQ1"], T["XQB"]
    XFB, GB, INDsb, INDT = T["XFB"], T["GB"], T["INDsb"], T["INDT"]
    W1SA1, W1SAB = T["w1sa1"], T["w1sab"]
    WPSA1, WPSAB = T["wpsa1"], T["wpsab"]
    W1CA1, W1CAB = T["w1ca1"], T["w1cab"]
    WPCA1, WPCAB = T["wpca1"], T["wpcab"]
    GW, DW, DB = T["GW"], T["DW"], T["DB"]
    ONES16W = T["ONES16W"]

    # per-body state (ring-buffered where consecutive bodies overlap)
    QA = res.tile([128, 2, QL], FP8, tag="qa", name="qa", bufs=2)
    QAB = res.tile([128, 2, QL], FP8, tag="qab", name="qab", bufs=2)
    XN1 = res.tile([128, 2, QL], FP8, tag="xn1", name="xn1", bufs=1)
    XNB = res.tile([128, 2, QL], FP8, tag="xnb", name="xnb", bufs=1)
    X3B = [res.tile([asz, QL], BF16, tag=f"x3b_{i}", name=f"x3b_{i}", bufs=2)
           for i, asz in enumerate([128, 128, 65])]
    X2 = [res.tile([sz, QL], F32, tag=f"x2_{i}", name=f"x2_{i}", bufs=1)
          for i, (_, sz) in enumerate(CT)]
    X3 = [res.tile([sz, QL], F32, tag=f"x3_{i}", name=f"x3_{i}", bufs=1)
          for i, (_, sz) in enumerate(CT)]
    YT = [res.tile([sz, QL], F32, tag=f"yt_{i}", name=f"yt_{i}", bufs=2)
          for i, (_, sz) in enumerate(CT)]
    nc.vector.memset(X3B[2][64:65, :], 1.0)
    if "pe_only" in _PROBE:
        PRB_ES = res.tile([128, 2, QWIN], FP8, tag="prb_es", name="prb_es", bufs=1)
        PRB_ATA = res.tile([128, 2, QWIN], FP8, tag="prb_ata", name="prb_ata", bufs=1)
        PRB_T = res.tile([128, QWIN], BF16, tag="prb_t", name="prb_t", bufs=1)
        nc.vector.memset(PRB_ES, 0.25)
        nc.vector.memset(PRB_ATA, 0.25)
        nc.vector.memset(PRB_T, 0.25)

    def q_proj(S1, SB, W1, WB):
        """QA/QAB <- fused-Q projection of (S1,SB); pure DoubleRow."""
        for qw in range(NQW):
            qsl = slice(qw * QWIN, (qw + 1) * QWIN)
            pq = ps_psc2.tile([128, 2, QWIN], F32, tag="psc2", name="psc2")
            for i in range(2):
                off = 128 * i
                nc.tensor.matmul(pq[:, i, :], W1[:, :, off:off + 128], S1[:, :, qsl],
                                 start=True, stop=False, perf_mode=DR,
                                 skip_group_check=True)
                nc.tensor.matmul(pq[:, i, :], WB[:, :, off:off + 128], SB[:, :, qsl],
                                 start=False, stop=True, perf_mode=DR,
                                 skip_group_check=True)
            pq2 = ps_tiny.tile([64, QWIN], F32, tag="tiny", name="tiny")
            nc.tensor.matmul(pq2, W1[:, :, 256:C], S1[:, :, qsl],
                             start=True, stop=False, perf_mode=DR,
                             skip_group_check=True)
            nc.tensor.matmul(pq2, WB[:, :, 256:C], SB[:, :, qsl],
                             start=False, stop=True, perf_mode=DR,
                             skip_group_check=True)
            nc.vector.tensor_copy(out=QA[:, :, qsl], in_=pq)
            nc.vector.tensor_copy(out=QAB[0:64, 0, qsl], in_=pq2)

    def attention_core(K1, K1B, TOK, WP1, WPB, resid_fn):
        """scores -> exp -> S@V(raw) -> normalize -> fused P; resid_fn(co,qw,pp).

        One-stage software pipeline: PE computes pair p+1 scores while ACT
        runs exp(p); S@V of pair p lands after scores of p+1 in PE order, so
        the in-order PE queue never waits on the exp round-trip."""
        def scores(qw, pair):
            qsl = slice(qw * QWIN, (qw + 1) * QWIN)
            es = res.tile([128, 2, QWIN], FP8, tag="es", name="es", bufs=4)
            psc = ps_psc2.tile([128, 2, QWIN], F32, tag="psc2", name="psc2")
            for g in range(2):
                mt = 2 * pair + g
                ksl = slice(mt * 128, (mt + 1) * 128)
                nc.tensor.matmul(psc[:, g, :], K1[:, :, ksl], QA[:, :, qsl],
                                 start=True, stop=False, perf_mode=DR,
                                 skip_group_check=True)
                nc.tensor.matmul(psc[:, g, :], K1B[:, :, ksl], QAB[:, :, qsl],
                                 start=False, stop=True, perf_mode=DR,
                                 skip_group_check=True)
            if "pe_only" in _PROBE:
                return PRB_ES
            nc.scalar.activation(out=es, in_=psc,
                                 func=mybir.ActivationFunctionType.Exp,
                                 scale=1.0 / QS)
            return es

        def sv(ctx, pair, es):
            po01, po2 = ctx["po01"], ctx["po2"]
            st, sp = (pair == 0), (pair == NPAIR - 1)
            for cj in range(2):
                off = CT[cj][0]
                nc.tensor.matmul(po01[:, cj, :],
                                 TOK[:, pair, :, off:off + 128], es,
                                 start=st, stop=sp,
                                 perf_mode=DR, skip_group_check=True)
            nc.tensor.matmul(po2, TOK[:, pair, :, 256:256 + ASZ[2]], es,
                             start=st, stop=sp,
                             perf_mode=DR, skip_group_check=True)

        def finisher(ctx):
            # softmax denominator -> normalize -> fused P projection
            qw, po01, po2 = ctx["qw"], ctx["po01"], ctx["po2"]
            if "pe_only" in _PROBE:
                pp = ps_psc2.tile([128, 2, QWIN], F32, tag="psc2", name="psc2")
                for co, (off, sz) in enumerate(CT):
                    ppv = (ps_tiny.tile([64, QWIN], F32, tag="tiny", name="tiny")
                           if co == 2 else pp[0:sz, co, :])
                    nc.tensor.matmul(ppv, WP1[:, :, off:off + sz], PRB_ATA,
                                     start=True, stop=False, perf_mode=DR,
                                     skip_group_check=True)
                    nc.tensor.matmul(ppv, WPB[:, :, off:off + sz], PRB_ATA,
                                     start=False, stop=True, perf_mode=DR,
                                     skip_group_check=True)
                    resid_fn(co, qw, ppv)
                return
            rec = res.tile([1, QWIN], F32, tag="rec", name="rec", bufs=2)
            nc.vector.reciprocal(rec, po2[64:65, :])
            dbc = res.tile([128, QWIN], F32, tag="dbc", name="dbc", bufs=2)
            nc.gpsimd.partition_broadcast(dbc, rec)
            ata = res.tile([128, 2, QWIN], FP8, tag="ata", name="ata", bufs=2)
            atb = res.tile([128, 2, QWIN], FP8, tag="atb", name="atb", bufs=2)
            nc.vector.tensor_mul(ata[:, 0, :], po01[:, 0, :], dbc)
            nc.vector.tensor_mul(ata[:, 1, :], po01[:, 1, :], dbc)
            nc.vector.tensor_mul(atb[0:65, 0, :], po2[0:65, :], dbc[0:65, :])
            pp = ps_psc2.tile([128, 2, QWIN], F32, tag="psc2", name="psc2")
            for co, (off, sz) in enumerate(CT):
                if co == 2:
                    ppv = ps_tiny.tile([64, QWIN], F32, tag="tiny", name="tiny")
                else:
                    ppv = pp[0:sz, co, :]
                nc.tensor.matmul(ppv, WP1[:, :, off:off + sz], ata,
                                 start=True, stop=False, perf_mode=DR,
                                 skip_group_check=True)
                nc.tensor.matmul(ppv, WPB[:, :, off:off + sz], atb,
                                 start=False, stop=True, perf_mode=DR,
                                 skip_group_check=True)
                resid_fn(co, qw, ppv)

        # Flat two-window pipeline. The window finisher (DVE-heavy softmax
        # normalization) is emitted AFTER the next window's first two score
        # pairs so the in-order PE queue always has data-ready matmuls.
        pend = None   # (ctx, es, next_sv_pair) of the previous window
        for qw in range(NQW):
            ctx = {"qw": qw,
                   "po01": ps_acc2.tile([128, 2, QWIN], F32, tag="acc2", name="acc2"),
                   "po2": ps_acc1.tile([ASZ[2], QWIN], F32, tag="acc1", name="acc1")}
            prev = scores(qw, 0)
            cur = scores(qw, 1)
            if pend is not None:
                finisher(pend)
            sv(ctx, 0, prev)
            prev = cur
            for pair in range(2, NPAIR):
                cur = scores(qw, pair)
                sv(ctx, pair - 1, prev)
                prev = cur
            sv(ctx, NPAIR - 1, prev)
            pend = ctx
        finisher(pend)

    # ======== attn1 (self-attention) ========
    q_proj(XQ1, XQB, W1SA1, W1SAB)
    if upto <= 1:
        return

    def resid1(co, qw, pp):
        qsl = slice(qw * QWIN, (qw + 1) * QWIN)
        # x2 = 2*x + attn1   (XFB holds 2*x in bf16; pp carries PS scale)
        nc.vector.scalar_tensor_tensor(
            out=X2[co][:, qsl], in0=pp, scalar=1.0 / PS, in1=XFB[co][:, qsl],
            op0=mybir.AluOpType.mult, op1=mybir.AluOpType.add)

    attention_core(XK1, XK1B, XTOK, WPSA1, WPSAB, resid1)
    if upto <= 2:
        return

    # ======== group-norm stats + AllReduce ========
    s12 = [res.tile([sz, 2], F32, tag=f"s12_{i}", name=f"s12_{i}", bufs=1)
           for i, (_, sz) in enumerate(CT)]
    scratch = res.tile([128, QL], F32, tag="scratch", name="scratch", bufs=1)
    for i, (_, sz) in enumerate(CT):
        nc.vector.reduce_sum(out=s12[i][:, 0:1], in_=X2[i], axis=mybir.AxisListType.X)
        nc.scalar.activation(
            out=scratch[0:sz, :], in_=X2[i],
            func=mybir.ActivationFunctionType.Square, accum_out=s12[i][:, 1:2])
    pg = ps_tiny.tile([GROUPS, 2], F32, tag="tiny", name="tiny")
    for i in range(3):
        nc.tensor.matmul(pg, INDsb[i], s12[i], start=(i == 0), stop=(i == 2))
    g12 = res.tile([GROUPS, 2], F32, tag="g12", name="g12", bufs=1)
    nc.vector.tensor_copy(out=g12, in_=pg)
    ccin = dram.tile([GROUPS, 2], F32, tag="ccin", name="ccin")
    ccout = dram.tile([GROUPS, 2], F32, tag="ccout", name="ccout")
    nc.sync.dma_start(out=ccin, in_=g12)
    if not globals().get("_SKIP_COLLECTIVE"):
        nc.gpsimd.collective_compute(
            "AllReduce", mybir.AluOpType.add,
            replica_groups=[[0, 1, 2, 3], [4, 5, 6, 7]],
            ins=[ccin.opt()], outs=[ccout.opt()])
    else:
        nc.sync.dma_start(out=ccout, in_=ccin)
    gg = res.tile([GROUPS, 2], F32, tag="gg", name="gg", bufs=1)
    nc.sync.dma_start(out=gg, in_=ccout)
    if upto <= 3:
        return

    # ======== finish group norm ========
    gtmp = res.tile([GROUPS, 4], F32, tag="gtmp", name="gtmp", bufs=1)
    grp2 = res.tile([GROUPS, 2], F32, tag="grp2", name="grp2", bufs=1)
    inv = 1.0 / GCNT
    nc.vector.tensor_scalar_mul(out=grp2[:, 1:2], in0=gg[:, 0:1], scalar1=inv)   # mean
    nc.vector.tensor_scalar_mul(out=gtmp[:, 0:1], in0=gg[:, 1:2], scalar1=inv)   # E[x^2]
    nc.vector.tensor_mul(gtmp[:, 1:2], grp2[:, 1:2], grp2[:, 1:2])               # mean^2
    nc.vector.tensor_sub(gtmp[:, 2:3], gtmp[:, 0:1], gtmp[:, 1:2])               # var
    epst = res.tile([GROUPS, 1], F32, tag="epst", name="epst", bufs=1)
    nc.vector.memset(epst, float(EPS))
    nc.scalar.activation(out=gtmp[:, 3:4], in_=gtmp[:, 2:3],
                         func=mybir.ActivationFunctionType.Sqrt, bias=epst)
    nc.vector.reciprocal(grp2[:, 0:1], gtmp[:, 3:4])                             # rstd
    for i, (off, sz) in enumerate(CT):
        pc = ps_tiny.tile([sz, 2], F32, tag="tiny", name="tiny")
        nc.tensor.matmul(pc, INDT[:, off:off + sz], grp2, start=True, stop=True)
        scs = res.tile([sz, 4], F32, tag=f"scs_{i}", name=f"scs_{i}", bufs=1)
        nc.vector.tensor_mul(scs[:, 0:1], pc[:, 0:1], GB[i][:, 0:1])     # scale=rstd*gamma
        nc.vector.tensor_mul(scs[:, 3:4], pc[:, 1:2], scs[:, 0:1])      # mean*scale
        nc.vector.tensor_sub(scs[:, 1:2], GB[i][:, 1:2], scs[:, 3:4])   # shift
        nc.vector.tensor_scalar_add(out=scs[:, 2:3], in0=scs[:, 0:1], scalar1=1.0)
        # xn (fp8, attn2 Q-projection input); SBUF-only ops go to Pool
        xn_out = XN1[:, i, :] if i < 2 else XNB[0:64, 0, :]
        nc.gpsimd.tensor_scalar(
            out=xn_out, in0=X2[i], scalar1=scs[:, 0:1], scalar2=scs[:, 1:2],
            op0=mybir.AluOpType.mult, op1=mybir.AluOpType.add)
        # x2 <- x2 + xn  (= x2*(1+scale) + shift), fp32, in place
        nc.gpsimd.tensor_scalar(
            out=X2[i], in0=X2[i], scalar1=scs[:, 2:3], scalar2=scs[:, 1:2],
            op0=mybir.AluOpType.mult, op1=mybir.AluOpType.add)

    # ======== attn2 (cross-attention on raw context keys/values) ========
    q_proj(XN1, XNB, W1CA1, W1CAB)

    def resid2(co, qw, pp):
        qsl = slice(qw * QWIN, (qw + 1) * QWIN)
        # x3 = (x2 + xn) + attn2
        nc.vector.scalar_tensor_tensor(
            out=X3[co][:, qsl], in0=pp, scalar=1.0 / PS, in1=X2[co][:, qsl],
            op0=mybir.AluOpType.mult, op1=mybir.AluOpType.add)

    attention_core(CK1, CK1B, CTOK, WPCA1, WPCAB, resid2)
    # FFN input in bf16 aug chunks (fp8 here costs ~7e-3 rel err)
    nc.gpsimd.tensor_copy(out=X3B[0], in_=X3[0])
    nc.gpsimd.tensor_copy(out=X3B[1], in_=X3[1])
    nc.gpsimd.tensor_copy(out=X3B[2][0:64, :], in_=X3[2])
    if upto <= 4:
        return

    # ======== GEGLU FFN ========
    for qw in range(NQW):
        qsl = slice(qw * QWIN, (qw + 1) * QWIN)
        py01 = ps_acc2.tile([128, 2, QWIN], F32, tag="acc2", name="acc2")
        py2 = ps_acc1.tile([64, QWIN], F32, tag="acc1", name="acc1")
        py = [py01[:, 0, :], py01[:, 1, :], py2]
        def geglu(hh):
            ha = slice(hh * 128, (hh + 1) * 128)
            hg = slice(C4 + hh * 128, C4 + (hh + 1) * 128)
            pag = ps_psc2.tile([128, 2, QWIN], F32, tag="psc2", name="psc2")
            for ci in range(3):
                nc.tensor.matmul(pag[:, 1, :], GW[ci][:, hg], X3B[ci][:, qsl],
                                 start=(ci == 0), stop=(ci == 2),
                                 skip_group_check=True)
            for ci in range(3):
                nc.tensor.matmul(pag[:, 0, :], GW[ci][:, ha], X3B[ci][:, qsl],
                                 start=(ci == 0), stop=(ci == 2),
                                 skip_group_check=True)
            if "pe_only" in _PROBE:
                return PRB_T
            sg = res.tile([128, QWIN], F32, tag="sg", name="sg", bufs=3)
            nc.scalar.activation(out=sg, in_=pag[:, 1, :],
                                 func=mybir.ActivationFunctionType.Sigmoid,
                                 scale=1.702)
            gsg = res.tile([128, QWIN], BF16, tag="gsg", name="gsg", bufs=3)
            nc.vector.tensor_mul(gsg, pag[:, 1, :], sg)
            t = res.tile([128, QWIN], BF16, tag="fft", name="fft", bufs=3)
            nc.vector.tensor_mul(t, pag[:, 0, :], gsg)
            return t

        def dense(hh, t):
            for co, (off, sz) in enumerate(CT):
                nc.tensor.matmul(py[co], DW[hh][:, off:off + sz], t,
                                 start=(hh == 0), stop=False)

        ts_ = [geglu(0), geglu(1)]
        for hh in range(2, NDP * 2):
            ts_.append(geglu(hh))
            dense(hh - 2, ts_[hh - 2])
        dense(NDP * 2 - 2, ts_[NDP * 2 - 2])
        dense(NDP * 2 - 1, ts_[NDP * 2 - 1])
        for co, (off, sz) in enumerate(CT):
            nc.tensor.matmul(py[co], DB[:, off:off + sz], ONES16W,
                             start=False, stop=True)
            nc.vector.tensor_add(YT[co][:, qsl], py[co], X3[co][:, qsl])

    if not accum_out:
        # real kernel: single plain output DMA
        for i, (off, sz) in enumerate(CT):
            nc.sync.dma_start(out=ap["yt"][off:off + sz, :], in_=YT[i])
    # timing builds (accum_out=True): no per-body yt DMA -- the tick chain
    # below transitively forces every body's compute.
    if "tick" in ap:
        tick = res.tile([1, 4], F32, tag="tick", name="tick", bufs=1)
        for i in range(3):
            nc.vector.tensor_copy(out=tick[0:1, i:i + 1],
                                  in_=YT[i][0:1, QL - 1:QL])
        nc.sync.dma_start(out=ap["tick"], in_=tick)


_SHAPES = {
    "xk1": ([128, 2, N], FP8), "xk1b": ([128, 2, N], FP8),
    "ck1": ([128, 2, N], FP8), "ck1b": ([128, 2, N], FP8),
    "xtok": ([128, NPAIR, 2, TOKW], FP8), "ctok": ([128, NPAIR, 2, TOKW], FP8),
    "xq1": ([128, 2, QL], FP8), "xqb": ([128, 2, QL], FP8),
    "xf16": ([C, QL], BF16),
    "w1sa1": ([128, 2, C], FP8), "w1sab": ([128, 2, C], FP8),
    "wpsa1": ([128, 2, C], FP8), "wpsab": ([128, 2, C], FP8),
    "w1ca1": ([128, 2, C], FP8), "w1cab": ([128, 2, C], FP8),
    "wpca1": ([128, 2, C], FP8), "wpcab": ([128, 2, C], FP8),
    "gw": ([C + 1, C8], BF16), "dw": ([C4, C], BF16), "db": ([1, C], BF16),
    "gb": ([C, 2], F32), "ind": ([C, GROUPS], F32), "indt": ([GROUPS, C], F32),
}


def _declare(nc, tick=False):
    ap = {}
    for name, (shape, dt) in _SHAPES.items():
        ap[name] = nc.dram_tensor(name, shape, dt, kind="ExternalInput").ap()
    ap["yt"] = nc.dram_tensor("yt", [C, QL], F32, kind="ExternalOutput").ap()
    if tick:
        ap["tick"] = nc.dram_tensor("tick", [1, 4], F32, kind="ExternalOutput").ap()
    return ap


def _build(rep=1, accum_out=False, tick=False, upto=99, skip_collective=False):
    key = (rep, accum_out, tick, upto, skip_collective)
    if key in _NC_CACHE:
        return _NC_CACHE[key]
    nc = bacc.Bacc("TRN2", target_bir_lowering=False, debug=False, num_devices=NCORES)
    ap = _declare(nc, tick=tick)
    if skip_collective:
        globals()["_SKIP_COLLECTIVE"] = True
    try:
        with tile.TileContext(nc) as tc:
            with (
                tc.tile_pool(name="res", bufs=1) as res,
                tc.tile_pool(name="acc2", bufs=1, space="PSUM") as acc2,
                tc.tile_pool(name="acc1", bufs=1, space="PSUM") as acc1,
                tc.tile_pool(name="psc2", bufs=2, space="PSUM") as psc2,
                tc.tile_pool(name="tiny", bufs=1, space="PSUM") as tiny,
                tc.tile_pool(name="dram", bufs=1, space="DRAM") as dram,
            ):
                pools = {"res": res, "acc2": acc2, "acc1": acc1,
                         "psc2": psc2, "tiny": tiny, "dram": dram}
                T = _emit_loads(nc, ap, pools)
                for _ in range(rep):
                    _emit_body(nc, tc, ap, T, pools, upto=upto, accum_out=accum_out)
    finally:
        globals()["_SKIP_COLLECTIVE"] = False
    nc.finalize()
    _NC_CACHE[key] = nc
    return nc


def _dr_split(w):
    """[R<=321, M] f32 -> ([128,2,M], [128,2,M]) fp8 pure-DR layout.

    First tile holds rows 0..255 interleaved; second holds rows 256.. in
    group 0 (rest zero) so the contraction is two K=128 DoubleRow matmuls."""
    m = w.shape[1]
    w1 = np.ascontiguousarray(
        w[0:256].reshape(2, 128, m).swapaxes(0, 1)).astype(fp8)
    wb = np.zeros((128, 2, m), np.float32)
    wb[0:w.shape[0] - 256, 0, :] = w[256:]
    return w1, wb.astype(fp8)


def _prep_inputs(inputs):
    """Host-side fusion, scaling, layout, and sharding for the 8 cores."""
    f32 = np.float32

    x = np.asarray(inputs["x"], f32).reshape(B, N, C)
    ctx = np.asarray(inputs["context"], f32).reshape(B, N, C)
    xt = np.ascontiguousarray(x.transpose(0, 2, 1))      # [B, C, N]
    ctxt = np.ascontiguousarray(ctx.transpose(0, 2, 1))

    def keys_split(xt_b):
        return _dr_split(xt_b[0:320])

    def tok_major(x_b):
        arr = x_b.reshape(NPAIR, 2, 128, C).transpose(2, 0, 1, 3)  # [128,16,2,C]
        out = np.zeros((128, NPAIR, 2, TOKW), f32)
        out[:, :, :, :C] = arr
        out[:, :, :, C] = 1.0
        return out.astype(fp8)

    xk = [keys_split(xt[b]) for b in range(B)]
    ck = [keys_split(ctxt[b]) for b in range(B)]
    xtok = [tok_major(x[b]) for b in range(B)]
    ctok = [tok_major(ctx[b]) for b in range(B)]

    def fuse_qk(wq, bq, wk):
        wq, bq, wk = np.asarray(wq, f32), np.asarray(bq, f32), np.asarray(wk, f32)
        w = np.concatenate([wq @ wk.T, (wk @ bq)[None, :]], axis=0)  # [321, 320]
        return _dr_split(w * (SCALE * QS))

    def fuse_vp(wv, bv, wp, bp):
        wv, bv = np.asarray(wv, f32), np.asarray(bv, f32)
        wp, bp = np.asarray(wp, f32), np.asarray(bp, f32)
        w = np.concatenate([wv @ wp, (bv @ wp + bp)[None, :]], axis=0)
        return _dr_split(w * PS)

    w1sa1, w1sab = fuse_qk(inputs["sa_q_w"], inputs["sa_q_b"], inputs["sa_k_w"])
    wpsa1, wpsab = fuse_vp(inputs["sa_v_w"], inputs["sa_v_b"],
                           inputs["sa_p_w"], inputs["sa_p_b"])
    w1ca1, w1cab = fuse_qk(inputs["ca_q_w"], inputs["ca_q_b"], inputs["ca_k_w"])
    wpca1, wpcab = fuse_vp(inputs["ca_v_w"], inputs["ca_v_b"],
                           inputs["ca_p_w"], inputs["ca_p_b"])

    gw = np.concatenate([np.asarray(inputs["geglu_w"], f32),
                         np.asarray(inputs["geglu_b"], f32)[None, :]],
                        axis=0).astype(bf16)
    dw = np.asarray(inputs["dense_w"], f32).astype(bf16)               # [1280, 320]
    db = np.asarray(inputs["dense_b"], f32).reshape(1, C).astype(bf16)

    gb = np.stack([np.asarray(inputs["ca_norm_g"], f32),
                   np.asarray(inputs["ca_norm_b"], f32)], axis=1)  # [C, 2]
    ind = np.zeros((C, GROUPS), f32)
    ind[np.arange(C), np.arange(C) // GSIZE] = 1.0
    indt = np.ascontiguousarray(ind.T)

    weights = {
        "w1sa1": w1sa1, "w1sab": w1sab, "wpsa1": wpsa1, "wpsab": wpsab,
        "w1ca1": w1ca1, "w1cab": w1cab, "wpca1": wpca1, "wpcab": wpcab,
        "gw": gw, "dw": dw, "db": db,
        "gb": gb, "ind": ind, "indt": indt,
    }

    in_maps = []
    for c in range(NCORES):
        b = c // 4
        q0 = (c % 4) * QL
        xq_loc = np.concatenate([xt[b][:, q0:q0 + QL], np.ones((1, QL), f32)],
                                axis=0)
        xq1, xqb = _dr_split(xq_loc)
        m = {
            "xk1": xk[b][0], "xk1b": xk[b][1],
            "ck1": ck[b][0], "ck1b": ck[b][1],
            "xtok": xtok[b], "ctok": ctok[b],
            "xq1": xq1, "xqb": xqb,
            "xf16": np.ascontiguousarray(2.0 * xt[b][:, q0:q0 + QL]).astype(bf16),
        }
        m.update(weights)
        in_maps.append(m)
    return in_maps


def kernel(**inputs):
    in_maps = _prep_inputs(inputs)
    nc = _build()
    res = run_bass_kernel_spmd(nc, in_maps, list(range(NCORES)))
    out = np.zeros((B, N, C), np.float32)
    for c in range(NCORES):
        b = c // 4
        q0 = (c % 4) * QL
        out[b, q0:q0 + QL, :] = res.results[c]["yt"].T
    return out.reshape(B, H, W, C)


def _build_single(rep=1, upto=99):
    """Single-core, collective-free variant for TimelineSim analysis."""
    nc = bacc.Bacc("TRN2", target_bir_lowering=False, debug=False, num_devices=1)
    ap = _declare(nc)
    globals()["_SKIP_COLLECTIVE"] = True
    try:
        with tile.TileContext(nc) as tc:
            with (
                tc.tile_pool(name="res", bufs=1) as res,
                tc.tile_pool(name="acc2", bufs=1, space="PSUM") as acc2,
                tc.tile_pool(name="acc1", bufs=1, space="PSUM") as acc1,
                tc.tile_pool(name="psc2", bufs=2, space="PSUM") as psc2,
                tc.tile_pool(name="tiny", bufs=1, space="PSUM") as tiny,
                tc.tile_pool(name="dram", bufs=1, space="DRAM") as dram,
            ):
                pools = {"res": res, "acc2": acc2, "acc1": acc1,
                         "psc2": psc2, "tiny": tiny, "dram": dram}
                T = _emit_loads(nc, ap, pools)
                for _ in range(rep):
                    _emit_body(nc, tc, ap, T, pools, upto=upto)
    finally:
        globals()["_SKIP_COLLECTIVE"] = False
    nc.finalize()
    return nc


# revision 17
# speedup vs baseline: 1.0053x; 1.0053x over previous
"""Trainium2 Bass kernel for nn_BasicTransformerBlock_12738873000028.

Strategy (8 NeuronCores): data-parallel over batch (2) x sequence-parallel over
query rows (4) => core c handles batch c//4, query rows [(c%4)*1024, +1024).

v2: fp8 (e4m3) DoubleRow matmuls + projection folding.

- K-projection is folded into the Q projection: scores = (x Wq + bq)·(k Wk + bk)
  = x (Wq Wk^T) k^T + (Wk bq)·k + const(n)  -- the bk term is constant per query
  and cancels in softmax, so keys are the RAW x / context (already resident in
  fp8), and the fused Q weight is Wq @ Wk.T with bias row Wk @ bq.
- V-projection is folded into the output projection: (sum_m s_m v_m) Wp =
  (sum_m s_m x_m)(Wv Wp) + (bv Wp + bp), so V is the raw x / context in
  token-major fp8 layout (host-prepped) with a ones column providing the
  softmax denominator; denominator * reciprocal == 1 doubles as the bias-row
  input for the fused P projection.
- All big matmuls are fp8 with DoubleRow perf mode over channel rows 0..255
  ([128,2] interleave) plus a plain fp8 matmul for the 64/65-row tail (small-K
  DoubleRow measured 3x slower than plain -- avoid).
- Weights are host-scaled (QK-fused x1024*C^-0.5, VP-fused x1024, FFN x64) to
  sit in fp8's normal range; descales fold into the exp scale, the softmax
  reciprocal path, and scalar_tensor_tensor residual updates. The fp32
  residual stream, group-norm statistics and the 16x2 AllReduce are unchanged.
"""

import numpy as np
import ml_dtypes

import concourse.bacc as bacc
import concourse.tile as tile
from concourse import mybir
from concourse.bass_utils import run_bass_kernel_spmd

bf16 = ml_dtypes.bfloat16
fp8 = ml_dtypes.float8_e4m3
F32 = mybir.dt.float32
BF16 = mybir.dt.bfloat16
FP8 = mybir.dt.float8e4
DR = mybir.MatmulPerfMode.DoubleRow

B, H, W, C = 2, 64, 64, 320
N = H * W                      # 4096 tokens per batch
NCORES = 8
QL = N // 4                    # 1024 local query rows per core
QWIN = 512                     # query window (fp32 PSUM bank = 512 floats)
NQW = QL // QWIN               # 2 windows
C8 = 8 * C                     # 2560
C4 = 4 * C                     # 1280
GROUPS, EPS = 16, 1e-3
GSIZE = C // GROUPS            # 20 channels per group
GCNT = float(N * GSIZE)        # elements per (batch, group)
MT = N // 128                  # 32 key tiles
NPAIR = MT // 2                # 16 key-tile pairs
NDP = C4 // 256                # 5 dense contraction pairs
SCALE = float(C) ** -0.5
QS = 1024.0                    # QK-fused weight scale
PS = 1024.0                    # VP-fused weight scale
WS = 64.0                      # FFN weight scale
TOKW = 336                     # channels + ones col + pad to 16-elem stride
                               # (dual-fp8 ldweights: group stride % 16 == 0)

# output channel chunks: (offset, size)
CT = [(0, 128), (128, 128), (256, 64)]
# accumulator partition sizes (chunk2 carries denominator row + pad; even for
# dual-fp8 ldweights restrictions)
ASZ = [128, 128, 66]

_NC_CACHE = {}
_PROBE = set()    # timing probes: "pe_only" strips ACT/DVE from hot paths


def _emit_loads(nc, ap, pools):
    """Resident loads: emitted ONCE per program; bodies only read these."""
    res = pools["res"]

    def rtile(shape, dtype, tag):
        return res.tile(shape, dtype, tag=tag, name=tag)

    T = {}
    # ---------------- resident loads ----------------
    # Pure-DoubleRow layout: contraction padded 320/321 -> 512 with zeros in
    # the "b" tiles (zero stationary rows make the pad free; avoids the
    # ~800ns/pair PE penalty for alternating dual-fp8 and plain matmuls).
    T["XK1"] = rtile([128, 2, N], FP8, "xk1")     # x channels 0..255, DR pairs
    T["XK1B"] = rtile([128, 2, N], FP8, "xk1b")   # chs 256..319 in g0 + zeros
    T["CK1"] = rtile([128, 2, N], FP8, "ck1")
    T["CK1B"] = rtile([128, 2, N], FP8, "ck1b")
    T["XTOK"] = rtile([128, NPAIR, 2, TOKW], FP8, "xtok")  # token-major + ones col
    T["CTOK"] = rtile([128, NPAIR, 2, TOKW], FP8, "ctok")
    T["XQ1"] = rtile([128, 2, QL], FP8, "xq1")    # local query cols
    T["XQB"] = rtile([128, 2, QL], FP8, "xqb")    # chs 256..319 + ones + zeros
    for name in ["xk1", "xk1b", "ck1", "ck1b", "xtok", "ctok", "xq1", "xqb"]:
        nc.sync.dma_start(out=T[name.upper()], in_=ap[name])

    def load_w(name, shape):
        t = rtile(shape, FP8, f"w_{name}")
        nc.sync.dma_start(out=t, in_=ap[name])
        return t

    T["XFB"], T["GB"], T["INDsb"] = [], [], []
    for i, (off, sz) in enumerate(CT):
        t = rtile([sz, QL], BF16, f"xfb_{i}")    # 2*x local, bf16 (resid1 base)
        nc.sync.dma_start(out=t, in_=ap["xf16"][off:off + sz, :])
        T["XFB"].append(t)
        t = rtile([sz, 2], F32, f"gb_{i}")
        nc.sync.dma_start(out=t, in_=ap["gb"][off:off + sz, :])
        T["GB"].append(t)
        t = rtile([sz, GROUPS], F32, f"ind_{i}")
        nc.sync.dma_start(out=t, in_=ap["ind"][off:off + sz, :])
        T["INDsb"].append(t)
    T["INDT"] = rtile([GROUPS, C], F32, "indt")
    nc.sync.dma_start(out=T["INDT"], in_=ap["indt"][:, :])


    for nm in ["w1sa1", "wpsa1", "w1ca1", "wpca1",
               "w1sab", "wpsab", "w1cab", "wpcab"]:
        T[nm] = load_w(nm, [128, 2, C])
    T["GW"] = []
    for i, asz in enumerate([128, 128, 65]):
        t = rtile([asz, C8], BF16, f"w_gw_{i}")
        nc.sync.dma_start(out=t, in_=ap["gw"][128 * i:128 * i + asz, :])
        T["GW"].append(t)
    T["DW"] = []
    for j in range(C4 // 128):
        t = rtile([128, C], BF16, f"w_dw_{j}")
        nc.sync.dma_start(out=t, in_=ap["dw"][j * 128:(j + 1) * 128, :])
        T["DW"].append(t)
    T["DB"] = rtile([1, C], BF16, "w_db")
    nc.sync.dma_start(out=T["DB"], in_=ap["db"])

    T["ONES16W"] = rtile([1, QWIN], BF16, "ones16w")
    nc.vector.memset(T["ONES16W"], 1.0)
    # pre-zero ring-buffered pad tiles (bodies only write their live rows,
    # the zero pad persists across bodies)
    for _ in range(2):
        t = res.tile([128, 2, QWIN], FP8, tag="atb", name="atb", bufs=2)
        nc.vector.memset(t, 0.0)
    for _ in range(2):
        t = res.tile([128, 2, QL], FP8, tag="qab", name="qab", bufs=2)
        nc.vector.memset(t, 0.0)
    t = res.tile([128, 2, QL], FP8, tag="xnb", name="xnb", bufs=1)
    nc.vector.memset(t, 0.0)
    nc.vector.memset(t[64:65, 0, :], 1.0)
    return T


def _emit_body(nc, tc, ap, T, pools, upto=99, accum_out=False):
    """Emit one forward pass reading resident tiles T."""
    res, dram = pools["res"], pools["dram"]
    ps_acc2, ps_acc1 = pools["acc2"], pools["acc1"]
    ps_psc2, ps_tiny = pools["psc2"], pools["tiny"]

    XK1, XK1B, CK1, CK1B = T["XK1"], T["XK1B"], T["CK1"], T["CK1B"]
    XTOK, CTOK, XQ1, XQB = T["XTOK"], T["CTOK"], T["XQ1"], T["XQB"]
    XFB, GB, INDsb, INDT = T["XFB"], T["GB"], T["INDsb"], T["INDT"]
    W1SA1, W1SAB = T["w1sa1"], T["w1sab"]
    WPSA1, WPSAB = T["wpsa1"], T["wpsab"]
    W1CA1, W1CAB = T["w1ca1"], T["w1cab"]
    WPCA1, WPCAB = T["wpca1"], T["wpcab"]
    GW, DW, DB = T["GW"], T["DW"], T["DB"]
    ONES16W = T["ONES16W"]

    # per-body state (ring-buffered where consecutive bodies overlap)
    QA = res.tile([128, 2, QL], FP8, tag="qa", name="qa", bufs=2)
    QAB = res.tile([128, 2, QL], FP8, tag="qab", name="qab", bufs=2)
    XN1 = res.tile([128, 2, QL], FP8, tag="xn1", name="xn1", bufs=1)
    XNB = res.tile([128, 2, QL], FP8, tag="xnb", name="xnb", bufs=1)
    X3B = [res.tile([asz, QL], BF16, tag=f"x3b_{i}", name=f"x3b_{i}", bufs=2)
           for i, asz in enumerate([128, 128, 65])]
    X2 = [res.tile([sz, QL], F32, tag=f"x2_{i}", name=f"x2_{i}", bufs=1)
          for i, (_, sz) in enumerate(CT)]
    X3 = [res.tile([sz, QL], F32, tag=f"x3_{i}", name=f"x3_{i}", bufs=1)
          for i, (_, sz) in enumerate(CT)]
    YT = [res.tile([sz, QL], F32, tag=f"yt_{i}", name=f"yt_{i}", bufs=2)
          for i, (_, sz) in enumerate(CT)]
    nc.vector.memset(X3B[2][64:65, :], 1.0)
    if "pe_only" in _PROBE:
        PRB_ES = res.tile([128, 2, QWIN], FP8, tag="prb_es", name="prb_es", bufs=1)
        PRB_ATA = res.tile([128, 2, QWIN], FP8, tag="prb_ata", name="prb_ata", bufs=1)
        PRB_T = res.tile([128, QWIN], BF16, tag="prb_t", name="prb_t", bufs=1)
        nc.vector.memset(PRB_ES, 0.25)
        nc.vector.memset(PRB_ATA, 0.25)
        nc.vector.memset(PRB_T, 0.25)

    def q_proj(S1, SB, W1, WB):
        """QA/QAB <- fused-Q projection of (S1,SB); pure DoubleRow."""
        for qw in range(NQW):
            qsl = slice(qw * QWIN, (qw + 1) * QWIN)
            pq = ps_psc2.tile([128, 2, QWIN], F32, tag="psc2", name="psc2")
            for i in range(2):
                off = 128 * i
                nc.tensor.matmul(pq[:, i, :], W1[:, :, off:off + 128], S1[:, :, qsl],
                                 start=True, stop=False, perf_mode=DR,
                                 skip_group_check=True)
                nc.tensor.matmul(pq[:, i, :], WB[:, :, off:off + 128], SB[:, :, qsl],
                                 start=False, stop=True, perf_mode=DR,
                                 skip_group_check=True)
            pq2 = ps_tiny.tile([64, QWIN], F32, tag="tiny", name="tiny")
            nc.tensor.matmul(pq2, W1[:, :, 256:C], S1[:, :, qsl],
                             start=True, stop=False, perf_mode=DR,
                             skip_group_check=True)
            nc.tensor.matmul(pq2, WB[:, :, 256:C], SB[:, :, qsl],
                             start=False, stop=True, perf_mode=DR,
                             skip_group_check=True)
            nc.vector.tensor_copy(out=QA[:, :, qsl], in_=pq)
            nc.vector.tensor_copy(out=QAB[0:64, 0, qsl], in_=pq2)

    def attention_core(K1, K1B, TOK, WP1, WPB, resid_fn):
        """scores -> exp -> S@V(raw) -> normalize -> fused P; resid_fn(co,qw,pp).

        One-stage software pipeline: PE computes pair p+1 scores while ACT
        runs exp(p); S@V of pair p lands after scores of p+1 in PE order, so
        the in-order PE queue never waits on the exp round-trip."""
        def scores(qw, pair):
            qsl = slice(qw * QWIN, (qw + 1) * QWIN)
            es = res.tile([128, 2, QWIN], FP8, tag="es", name="es", bufs=3)
            psc = ps_psc2.tile([128, 2, QWIN], F32, tag="psc2", name="psc2")
            for g in range(2):
                mt = 2 * pair + g
                ksl = slice(mt * 128, (mt + 1) * 128)
                nc.tensor.matmul(psc[:, g, :], K1[:, :, ksl], QA[:, :, qsl],
                                 start=True, stop=False, perf_mode=DR,
                                 skip_group_check=True)
                nc.tensor.matmul(psc[:, g, :], K1B[:, :, ksl], QAB[:, :, qsl],
                                 start=False, stop=True, perf_mode=DR,
                                 skip_group_check=True)
            if "pe_only" in _PROBE:
                return PRB_ES
            nc.scalar.activation(out=es, in_=psc,
                                 func=mybir.ActivationFunctionType.Exp,
                                 scale=1.0 / QS)
            return es

        def sv(ctx, pair, es):
            po01, po2 = ctx["po01"], ctx["po2"]
            st, sp = (pair == 0), (pair == NPAIR - 1)
            for cj in range(2):
                off = CT[cj][0]
                nc.tensor.matmul(po01[:, cj, :],
                                 TOK[:, pair, :, off:off + 128], es,
                                 start=st, stop=sp,
                                 perf_mode=DR, skip_group_check=True)
            nc.tensor.matmul(po2, TOK[:, pair, :, 256:256 + ASZ[2]], es,
                             start=st, stop=sp,
                             perf_mode=DR, skip_group_check=True)

        def finisher(ctx):
            # softmax denominator -> normalize -> fused P projection
            qw, po01, po2 = ctx["qw"], ctx["po01"], ctx["po2"]
            if "pe_only" in _PROBE:
                pp = ps_psc2.tile([128, 2, QWIN], F32, tag="psc2", name="psc2")
                for co, (off, sz) in enumerate(CT):
                    ppv = (ps_tiny.tile([64, QWIN], F32, tag="tiny", name="tiny")
                           if co == 2 else pp[0:sz, co, :])
                    nc.tensor.matmul(ppv, WP1[:, :, off:off + sz], PRB_ATA,
                                     start=True, stop=False, perf_mode=DR,
                                     skip_group_check=True)
                    nc.tensor.matmul(ppv, WPB[:, :, off:off + sz], PRB_ATA,
                                     start=False, stop=True, perf_mode=DR,
                                     skip_group_check=True)
                    resid_fn(co, qw, ppv)
                return
            rec = res.tile([1, QWIN], F32, tag="rec", name="rec", bufs=2)
            nc.vector.reciprocal(rec, po2[64:65, :])
            dbc = res.tile([128, QWIN], F32, tag="dbc", name="dbc", bufs=2)
            nc.gpsimd.partition_broadcast(dbc, rec)
            ata = res.tile([128, 2, QWIN], FP8, tag="ata", name="ata", bufs=2)
            atb = res.tile([128, 2, QWIN], FP8, tag="atb", name="atb", bufs=2)
            nc.vector.tensor_mul(ata[:, 0, :], po01[:, 0, :], dbc)
            nc.vector.tensor_mul(ata[:, 1, :], po01[:, 1, :], dbc)
            nc.vector.tensor_mul(atb[0:65, 0, :], po2[0:65, :], dbc[0:65, :])
            pp = ps_psc2.tile([128, 2, QWIN], F32, tag="psc2", name="psc2")
            for co, (off, sz) in enumerate(CT):
                if co == 2:
                    ppv = ps_tiny.tile([64, QWIN], F32, tag="tiny", name="tiny")
                else:
                    ppv = pp[0:sz, co, :]
                nc.tensor.matmul(ppv, WP1[:, :, off:off + sz], ata,
                                 start=True, stop=False, perf_mode=DR,
                                 skip_group_check=True)
                nc.tensor.matmul(ppv, WPB[:, :, off:off + sz], atb,
                                 start=False, stop=True, perf_mode=DR,
                                 skip_group_check=True)
                resid_fn(co, qw, ppv)

        # Flat two-window pipeline. The window finisher (DVE-heavy softmax
        # normalization) is emitted AFTER the next window's first two score
        # pairs so the in-order PE queue always has data-ready matmuls.
        pend = None   # (ctx, es, next_sv_pair) of the previous window
        for qw in range(NQW):
            ctx = {"qw": qw,
                   "po01": ps_acc2.tile([128, 2, QWIN], F32, tag="acc2", name="acc2"),
                   "po2": ps_acc1.tile([ASZ[2], QWIN], F32, tag="acc1", name="acc1")}
            prev = scores(qw, 0)
            cur = scores(qw, 1)
            if pend is not None:
                finisher(pend)
            sv(ctx, 0, prev)
            prev = cur
            for pair in range(2, NPAIR):
                cur = scores(qw, pair)
                sv(ctx, pair - 1, prev)
                prev = cur
            sv(ctx, NPAIR - 1, prev)
            pend = ctx
        finisher(pend)

    # ======== attn1 (self-attention) ========
    q_proj(XQ1, XQB, W1SA1, W1SAB)
    if upto <= 1:
        return

    def resid1(co, qw, pp):
        qsl = slice(qw * QWIN, (qw + 1) * QWIN)
        # x2 = 2*x + attn1   (XFB holds 2*x in bf16; pp carries PS scale)
        nc.vector.scalar_tensor_tensor(
            out=X2[co][:, qsl], in0=pp, scalar=1.0 / PS, in1=XFB[co][:, qsl],
            op0=mybir.AluOpType.mult, op1=mybir.AluOpType.add)

    attention_core(XK1, XK1B, XTOK, WPSA1, WPSAB, resid1)
    if upto <= 2:
        return

    # ======== group-norm stats + AllReduce ========
    s12 = [res.tile([sz, 2], F32, tag=f"s12_{i}", name=f"s12_{i}", bufs=1)
           for i, (_, sz) in enumerate(CT)]
    scratch = res.tile([128, QL], F32, tag="scratch", name="scratch", bufs=1)
    for i, (_, sz) in enumerate(CT):
        nc.vector.reduce_sum(out=s12[i][:, 0:1], in_=X2[i], axis=mybir.AxisListType.X)
        nc.scalar.activation(
            out=scratch[0:sz, :], in_=X2[i],
            func=mybir.ActivationFunctionType.Square, accum_out=s12[i][:, 1:2])
    pg = ps_tiny.tile([GROUPS, 2], F32, tag="tiny", name="tiny")
    for i in range(3):
        nc.tensor.matmul(pg, INDsb[i], s12[i], start=(i == 0), stop=(i == 2))
    g12 = res.tile([GROUPS, 2], F32, tag="g12", name="g12", bufs=1)
    nc.vector.tensor_copy(out=g12, in_=pg)
    ccin = dram.tile([GROUPS, 2], F32, tag="ccin", name="ccin")
    ccout = dram.tile([GROUPS, 2], F32, tag="ccout", name="ccout")
    nc.sync.dma_start(out=ccin, in_=g12)
    if not globals().get("_SKIP_COLLECTIVE"):
        nc.gpsimd.collective_compute(
            "AllReduce", mybir.AluOpType.add,
            replica_groups=[[0, 1, 2, 3], [4, 5, 6, 7]],
            ins=[ccin.opt()], outs=[ccout.opt()])
    else:
        nc.sync.dma_start(out=ccout, in_=ccin)
    gg = res.tile([GROUPS, 2], F32, tag="gg", name="gg", bufs=1)
    nc.sync.dma_start(out=gg, in_=ccout)
    if upto <= 3:
        return

    # ======== finish group norm ========
    gtmp = res.tile([GROUPS, 4], F32, tag="gtmp", name="gtmp", bufs=1)
    grp2 = res.tile([GROUPS, 2], F32, tag="grp2", name="grp2", bufs=1)
    inv = 1.0 / GCNT
    nc.vector.tensor_scalar_mul(out=grp2[:, 1:2], in0=gg[:, 0:1], scalar1=inv)   # mean
    nc.vector.tensor_scalar_mul(out=gtmp[:, 0:1], in0=gg[:, 1:2], scalar1=inv)   # E[x^2]
    nc.vector.tensor_mul(gtmp[:, 1:2], grp2[:, 1:2], grp2[:, 1:2])               # mean^2
    nc.vector.tensor_sub(gtmp[:, 2:3], gtmp[:, 0:1], gtmp[:, 1:2])               # var
    epst = res.tile([GROUPS, 1], F32, tag="epst", name="epst", bufs=1)
    nc.vector.memset(epst, float(EPS))
    nc.scalar.activation(out=gtmp[:, 3:4], in_=gtmp[:, 2:3],
                         func=mybir.ActivationFunctionType.Sqrt, bias=epst)
    nc.vector.reciprocal(grp2[:, 0:1], gtmp[:, 3:4])                             # rstd
    for i, (off, sz) in enumerate(CT):
        pc = ps_tiny.tile([sz, 2], F32, tag="tiny", name="tiny")
        nc.tensor.matmul(pc, INDT[:, off:off + sz], grp2, start=True, stop=True)
        scs = res.tile([sz, 4], F32, tag=f"scs_{i}", name=f"scs_{i}", bufs=1)
        nc.vector.tensor_mul(scs[:, 0:1], pc[:, 0:1], GB[i][:, 0:1])     # scale=rstd*gamma
        nc.vector.tensor_mul(scs[:, 3:4], pc[:, 1:2], scs[:, 0:1])      # mean*scale
        nc.vector.tensor_sub(scs[:, 1:2], GB[i][:, 1:2], scs[:, 3:4])   # shift
        nc.vector.tensor_scalar_add(out=scs[:, 2:3], in0=scs[:, 0:1], scalar1=1.0)
        # xn (fp8, attn2 Q-projection input); SBUF-only ops go to Pool
        xn_out = XN1[:, i, :] if i < 2 else XNB[0:64, 0, :]
        nc.gpsimd.tensor_scalar(
            out=xn_out, in0=X2[i], scalar1=scs[:, 0:1], scalar2=scs[:, 1:2],
            op0=mybir.AluOpType.mult, op1=mybir.AluOpType.add)
        # x2 <- x2 + xn  (= x2*(1+scale) + shift), fp32, in place
        nc.gpsimd.tensor_scalar(
            out=X2[i], in0=X2[i], scalar1=scs[:, 2:3], scalar2=scs[:, 1:2],
            op0=mybir.AluOpType.mult, op1=mybir.AluOpType.add)

    # ======== attn2 (cross-attention on raw context keys/values) ========
    q_proj(XN1, XNB, W1CA1, W1CAB)

    def resid2(co, qw, pp):
        qsl = slice(qw * QWIN, (qw + 1) * QWIN)
        # x3 = (x2 + xn) + attn2
        nc.vector.scalar_tensor_tensor(
            out=X3[co][:, qsl], in0=pp, scalar=1.0 / PS, in1=X2[co][:, qsl],
            op0=mybir.AluOpType.mult, op1=mybir.AluOpType.add)

    attention_core(CK1, CK1B, CTOK, WPCA1, WPCAB, resid2)
    # FFN input in bf16 aug chunks (fp8 here costs ~7e-3 rel err)
    nc.gpsimd.tensor_copy(out=X3B[0], in_=X3[0])
    nc.gpsimd.tensor_copy(out=X3B[1], in_=X3[1])
    nc.gpsimd.tensor_copy(out=X3B[2][0:64, :], in_=X3[2])
    if upto <= 4:
        return

    # ======== GEGLU FFN ========
    for qw in range(NQW):
        qsl = slice(qw * QWIN, (qw + 1) * QWIN)
        py01 = ps_acc2.tile([128, 2, QWIN], F32, tag="acc2", name="acc2")
        py2 = ps_acc1.tile([64, QWIN], F32, tag="acc1", name="acc1")
        py = [py01[:, 0, :], py01[:, 1, :], py2]
        def geglu(hh):
            ha = slice(hh * 128, (hh + 1) * 128)
            hg = slice(C4 + hh * 128, C4 + (hh + 1) * 128)
            pag = ps_psc2.tile([128, 2, QWIN], F32, tag="psc2", name="psc2")
            for ci in range(3):
                nc.tensor.matmul(pag[:, 1, :], GW[ci][:, hg], X3B[ci][:, qsl],
                                 start=(ci == 0), stop=(ci == 2),
                                 skip_group_check=True)
            for ci in range(3):
                nc.tensor.matmul(pag[:, 0, :], GW[ci][:, ha], X3B[ci][:, qsl],
                                 start=(ci == 0), stop=(ci == 2),
                                 skip_group_check=True)
            if "pe_only" in _PROBE:
                return PRB_T
            sg = res.tile([128, QWIN], F32, tag="sg", name="sg", bufs=2)
            nc.scalar.activation(out=sg, in_=pag[:, 1, :],
                                 func=mybir.ActivationFunctionType.Sigmoid,
                                 scale=1.702)
            gsg = res.tile([128, QWIN], BF16, tag="gsg", name="gsg", bufs=2)
            nc.vector.tensor_mul(gsg, pag[:, 1, :], sg)
            t = res.tile([128, QWIN], BF16, tag="fft", name="fft", bufs=3)
            nc.vector.tensor_mul(t, pag[:, 0, :], gsg)
            return t

        def dense(hh, t):
            for co, (off, sz) in enumerate(CT):
                nc.tensor.matmul(py[co], DW[hh][:, off:off + sz], t,
                                 start=(hh == 0), stop=False)

        ts_ = [geglu(0), geglu(1)]
        for hh in range(2, NDP * 2):
            ts_.append(geglu(hh))
            dense(hh - 2, ts_[hh - 2])
        dense(NDP * 2 - 2, ts_[NDP * 2 - 2])
        dense(NDP * 2 - 1, ts_[NDP * 2 - 1])
        for co, (off, sz) in enumerate(CT):
            nc.tensor.matmul(py[co], DB[:, off:off + sz], ONES16W,
                             start=False, stop=True)
            nc.vector.tensor_add(YT[co][:, qsl], py[co], X3[co][:, qsl])

    if not accum_out:
        # real kernel: single plain output DMA
        for i, (off, sz) in enumerate(CT):
            nc.sync.dma_start(out=ap["yt"][off:off + sz, :], in_=YT[i])
    # timing builds (accum_out=True): no per-body yt DMA -- the tick chain
    # below transitively forces every body's compute.
    if "tick" in ap:
        tick = res.tile([1, 4], F32, tag="tick", name="tick", bufs=1)
        for i in range(3):
            nc.vector.tensor_copy(out=tick[0:1, i:i + 1],
                                  in_=YT[i][0:1, QL - 1:QL])
        nc.sync.dma_start(out=ap["tick"], in_=tick)


_SHAPES = {
    "xk1": ([128, 2, N], FP8), "xk1b": ([128, 2, N], FP8),
    "ck1": ([128, 2, N], FP8), "ck1b": ([128, 2, N], FP8),
    "xtok": ([128, NPAIR, 2, TOKW], FP8), "ctok": ([128, NPAIR, 2, TOKW], FP8),
    "xq1": ([128, 2, QL], FP8), "xqb": ([128, 2, QL], FP8),
    "xf16": ([C, QL], BF16),
    "w1sa1": ([128, 2, C], FP8), "w1sab": ([128, 2, C], FP8),
    "wpsa1": ([128, 2, C], FP8), "wpsab": ([128, 2, C], FP8),
    "w1ca1": ([128, 2, C], FP8), "w1cab": ([128, 2, C], FP8),
    "wpca1": ([128, 2, C], FP8), "wpcab": ([128, 2, C], FP8),
    "gw": ([C + 1, C8], BF16), "dw": ([C4, C], BF16), "db": ([1, C], BF16),
    "gb": ([C, 2], F32), "ind": ([C, GROUPS], F32), "indt": ([GROUPS, C], F32),
}


def _declare(nc, tick=False):
    ap = {}
    for name, (shape, dt) in _SHAPES.items():
        ap[name] = nc.dram_tensor(name, shape, dt, kind="ExternalInput").ap()
    ap["yt"] = nc.dram_tensor("yt", [C, QL], F32, kind="ExternalOutput").ap()
    if tick:
        ap["tick"] = nc.dram_tensor("tick", [1, 4], F32, kind="ExternalOutput").ap()
    return ap


def _build(rep=1, accum_out=False, tick=False, upto=99, skip_collective=False):
    key = (rep, accum_out, tick, upto, skip_collective)
    if key in _NC_CACHE:
        return _NC_CACHE[key]
    nc = bacc.Bacc("TRN2", target_bir_lowering=False, debug=False, num_devices=NCORES)
    ap = _declare(nc, tick=tick)
    if skip_collective:
        globals()["_SKIP_COLLECTIVE"] = True
    try:
        with tile.TileContext(nc) as tc:
            with (
                tc.tile_pool(name="res", bufs=1) as res,
                tc.tile_pool(name="acc2", bufs=1, space="PSUM") as acc2,
                tc.tile_pool(name="acc1", bufs=1, space="PSUM") as acc1,
                tc.tile_pool(name="psc2", bufs=2, space="PSUM") as psc2,
                tc.tile_pool(name="tiny", bufs=1, space="PSUM") as tiny,
                tc.tile_pool(name="dram", bufs=1, space="DRAM") as dram,
            ):
                pools = {"res": res, "acc2": acc2, "acc1": acc1,
                         "psc2": psc2, "tiny": tiny, "dram": dram}
                T = _emit_loads(nc, ap, pools)
                for _ in range(rep):
                    _emit_body(nc, tc, ap, T, pools, upto=upto, accum_out=accum_out)
    finally:
        globals()["_SKIP_COLLECTIVE"] = False
    nc.finalize()
    _NC_CACHE[key] = nc
    return nc


def _dr_split(w):
    """[R<=321, M] f32 -> ([128,2,M], [128,2,M]) fp8 pure-DR layout.

    First tile holds rows 0..255 interleaved; second holds rows 256.. in
    group 0 (rest zero) so the contraction is two K=128 DoubleRow matmuls."""
    m = w.shape[1]
    w1 = np.ascontiguousarray(
        w[0:256].reshape(2, 128, m).swapaxes(0, 1)).astype(fp8)
    wb = np.zeros((128, 2, m), np.float32)
    wb[0:w.shape[0] - 256, 0, :] = w[256:]
    return w1, wb.astype(fp8)


def _prep_inputs(inputs):
    """Host-side fusion, scaling, layout, and sharding for the 8 cores."""
    f32 = np.float32

    x = np.asarray(inputs["x"], f32).reshape(B, N, C)
    ctx = np.asarray(inputs["context"], f32).reshape(B, N, C)
    xt = np.ascontiguousarray(x.transpose(0, 2, 1))      # [B, C, N]
    ctxt = np.ascontiguousarray(ctx.transpose(0, 2, 1))

    def keys_split(xt_b):
        return _dr_split(xt_b[0:320])

    def tok_major(x_b):
        arr = x_b.reshape(NPAIR, 2, 128, C).transpose(2, 0, 1, 3)  # [128,16,2,C]
        out = np.zeros((128, NPAIR, 2, TOKW), f32)
        out[:, :, :, :C] = arr
        out[:, :, :, C] = 1.0
        return out.astype(fp8)

    xk = [keys_split(xt[b]) for b in range(B)]
    ck = [keys_split(ctxt[b]) for b in range(B)]
    xtok = [tok_major(x[b]) for b in range(B)]
    ctok = [tok_major(ctx[b]) for b in range(B)]

    def fuse_qk(wq, bq, wk):
        wq, bq, wk = np.asarray(wq, f32), np.asarray(bq, f32), np.asarray(wk, f32)
        w = np.concatenate([wq @ wk.T, (wk @ bq)[None, :]], axis=0)  # [321, 320]
        return _dr_split(w * (SCALE * QS))

    def fuse_vp(wv, bv, wp, bp):
        wv, bv = np.asarray(wv, f32), np.asarray(bv, f32)
        wp, bp = np.asarray(wp, f32), np.asarray(bp, f32)
        w = np.concatenate([wv @ wp, (bv @ wp + bp)[None, :]], axis=0)
        return _dr_split(w * PS)

    w1sa1, w1sab = fuse_qk(inputs["sa_q_w"], inputs["sa_q_b"], inputs["sa_k_w"])
    wpsa1, wpsab = fuse_vp(inputs["sa_v_w"], inputs["sa_v_b"],
                           inputs["sa_p_w"], inputs["sa_p_b"])
    w1ca1, w1cab = fuse_qk(inputs["ca_q_w"], inputs["ca_q_b"], inputs["ca_k_w"])
    wpca1, wpcab = fuse_vp(inputs["ca_v_w"], inputs["ca_v_b"],
                           inputs["ca_p_w"], inputs["ca_p_b"])

    gw = np.concatenate([np.asarray(inputs["geglu_w"], f32),
                         np.asarray(inputs["geglu_b"], f32)[None, :]],
                        axis=0).astype(bf16)
    dw = np.asarray(inputs["dense_w"], f32).astype(bf16)               # [1280, 320]
    db = np.asarray(inputs["dense_b"], f32).reshape(1, C).astype(bf16)

    gb = np.stack([np.asarray(inputs["ca_norm_g"], f32),
                   np.asarray(inputs["ca_norm_b"], f32)], axis=1)  # [C, 2]
    ind = np.zeros((C, GROUPS), f32)
    ind[np.arange(C), np.arange(C) // GSIZE] = 1.0
    indt = np.ascontiguousarray(ind.T)

    weights = {
        "w1sa1": w1sa1, "w1sab": w1sab, "wpsa1": wpsa1, "wpsab": wpsab,
        "w1ca1": w1ca1, "w1cab": w1cab, "wpca1": wpca1, "wpcab": wpcab,
        "gw": gw, "dw": dw, "db": db,
        "gb": gb, "ind": ind, "indt": indt,
    }

    in_maps = []
    for c in range(NCORES):
        b = c // 4
        q0 = (c % 4) * QL
        xq_loc = np.concatenate([xt[b][:, q0:q0 + QL], np.ones((1, QL), f32)],
                                axis=0)
        xq1, xqb = _dr_split(xq_loc)
        m = {
            "xk1": xk[b][0], "xk1b": xk[b][1],
            "ck1": ck[b][0], "ck1b": ck[b][1],
            "xtok": xtok[b], "ctok": ctok[b],
            "xq1": xq1, "xqb": xqb,
            "xf16": np.ascontiguousarray(2.0 * xt[b][:, q0:q0 + QL]).astype(bf16),
        }
        m.update(weights)
        in_maps.append(m)
    return in_maps


def kernel(**inputs):
    in_maps = _prep_inputs(inputs)
    nc = _build()
    res = run_bass_kernel_spmd(nc, in_maps, list(range(NCORES)))
    out = np.zeros((B, N, C), np.float32)
    for c in range(NCORES):
        b = c // 4
        q0 = (c % 4) * QL
        out[b, q0:q0 + QL, :] = res.results[c]["yt"].T
    return out.reshape(B, H, W, C)


def _build_single(rep=1, upto=99):
    """Single-core, collective-free variant for TimelineSim analysis."""
    nc = bacc.Bacc("TRN2", target_bir_lowering=False, debug=False, num_devices=1)
    ap = _declare(nc)
    globals()["_SKIP_COLLECTIVE"] = True
    try:
        with tile.TileContext(nc) as tc:
            with (
                tc.tile_pool(name="res", bufs=1) as res,
                tc.tile_pool(name="acc2", bufs=1, space="PSUM") as acc2,
                tc.tile_pool(name="acc1", bufs=1, space="PSUM") as acc1,
                tc.tile_pool(name="psc2", bufs=2, space="PSUM") as psc2,
                tc.tile_pool(name="tiny", bufs=1, space="PSUM") as tiny,
                tc.tile_pool(name="dram", bufs=1, space="DRAM") as dram,
            ):
                pools = {"res": res, "acc2": acc2, "acc1": acc1,
                         "psc2": psc2, "tiny": tiny, "dram": dram}
                T = _emit_loads(nc, ap, pools)
                for _ in range(rep):
                    _emit_body(nc, tc, ap, T, pools, upto=upto)
    finally:
        globals()["_SKIP_COLLECTIVE"] = False
    nc.finalize()
    return nc
